# revision 2
# baseline (speedup 1.0000x reference)
"""LorentzNodeBlock — hand-written Bass kernel for 8 TRN2 NeuronCores.

Sharding: nodes are sharded across the 8 cores (12500 destination nodes
per core); each core consumes exactly the edges destined to its nodes,
so no collective is needed.  The host performs index-only preprocessing:
it ranks every edge within its destination node and packs the per-edge
features [q(x[row]), edge_attr] into a fixed-geometry bf16 grid of
D=96 edge slots per node (pad slots hold a sentinel feature vector that
the edge MLP relu()s to exactly zero).  All FLOPs — edge MLP, segmented
mean, and node MLP — run on the NeuronCores.

Device program (per core, identical SPMD NEFF):
  grid [120, 150528] bf16  (rows 15k+j: j=0 -> q-values, j=1..14 ->
                            edge_attr features; k = node pack 0..7)
  14 blocks x (DMA 2.6MB; 21 matmuls w/ reused block-diag stationary;
               relu+bias on ACT/DVE; bf16 log-fold + f32 reduce -> S)
  epilogue: mean = (S @ w1b) * (1/count) + b1b   (matmul + DVE + ACT)
  node MLP: two block-diag matmul layers -> out [112, 1568] f32

Nodes with degree > 96 or degree == 0 (none for the target distribution,
but handled for safety) are recomputed exactly on the host.
"""

from contextlib import ExitStack

import numpy as np
import ml_dtypes

# ---- problem geometry (nn_LorentzNodeBlock: N=100000, HID=14) ----
N = 100000
HID = 14
NCORES = 8
NPN = 12500           # actual nodes per core
PACKS = 8
NPP = 1568            # node slots per pack
NLOC = PACKS * NPP    # 12544 node slots per core
D = 96                # edge slots per node
BS = 112              # nodes per block (per pack)
NBLK = NPP // BS      # 14
BSLOTS = BS * D       # 10752
ROWS = 15             # 1 q row + 14 edge_attr rows
GRID_W = NPP * D      # 150528
MM_F = 512
MM_PER_BLK = BSLOTS // MM_F  # 21

_MINK = np.array([-1.0, 1.0, 1.0, 1.0], dtype=np.float32)
_STATE = {}


def _bf16_bits(a):
    """float32 -> bfloat16 bit pattern (uint16), round-to-nearest-even."""
    u = np.ascontiguousarray(a, dtype=np.float32).view(np.uint32)
    one = np.uint32(1)
    r = (u + np.uint32(0x7FFF) + ((u >> np.uint32(16)) & one)) >> np.uint32(16)
    return r.astype(np.uint16)


def _bf16_val(a):
    return _bf16_bits(a).view(ml_dtypes.bfloat16)


# ======================= device kernel (Bass/Tile) =======================

def _build_nc(dve_relu_frac=3, dma_chunks=4):
    import concourse.tile as tile
    from concourse import bacc, mybir

    F32 = mybir.dt.float32
    BF16 = mybir.dt.bfloat16

    nc = bacc.Bacc(
        "TRN2",
        target_bir_lowering=False,
        debug=False,
        enable_asserts=False,
        num_devices=NCORES,
    )

    grid = nc.dram_tensor("grid", [ROWS * PACKS, GRID_W], BF16,
                          kind="ExternalInput").ap()
    invc = nc.dram_tensor("invc", [HID * PACKS, NPP], F32,
                          kind="ExternalInput").ap()
    qn = nc.dram_tensor("qn", [PACKS, NPP], F32, kind="ExternalInput").ap()
    w1t = nc.dram_tensor("w1t", [ROWS * PACKS, HID * PACKS], BF16,
                         kind="ExternalInput").ap()
    b1 = nc.dram_tensor("b1", [HID * PACKS, 1], F32,
                        kind="ExternalInput").ap()
    w1bt = nc.dram_tensor("w1bt", [HID * PACKS, HID * PACKS], F32,
                          kind="ExternalInput").ap()
    b1b = nc.dram_tensor("b1b", [HID * PACKS, 1], F32,
                         kind="ExternalInput").ap()
    w2t = nc.dram_tensor("w2t", [16 * PACKS, HID * PACKS], F32,
                         kind="ExternalInput").ap()
    b2 = nc.dram_tensor("b2", [HID * PACKS, 1], F32,
                        kind="ExternalInput").ap()
    w3t = nc.dram_tensor("w3t", [HID * PACKS, HID * PACKS], F32,
                         kind="ExternalInput").ap()
    b3 = nc.dram_tensor("b3", [HID * PACKS, 1], F32,
                        kind="ExternalInput").ap()
    out = nc.dram_tensor("out", [HID * PACKS, NPP], F32,
                         kind="ExternalOutput").ap()

    with ExitStack() as ctx:
        tc = ctx.enter_context(tile.TileContext(nc))
        _kernel_body(nc, tc, ctx, mybir, F32, BF16,
                     grid, invc, qn, w1t, b1, w1bt, b1b, w2t, b2, w3t, b3,
                     out, dve_relu_frac, dma_chunks)

    nc.compile()
    return nc


def _kernel_body(nc, tc, ctx, mybir, F32, BF16, grid, invc, qn, w1t, b1,
                 w1bt, b1b, w2t, b2, w3t, b3, out, dve_relu_frac,
                 dma_chunks):
    const = ctx.enter_context(tc.tile_pool(name="const", bufs=1))
    rhs_pool = ctx.enter_context(tc.tile_pool(name="rhs", bufs=3))
    psum_pool = ctx.enter_context(
        tc.tile_pool(name="psum", bufs=4, space="PSUM"))
    r_pool = ctx.enter_context(tc.tile_pool(name="r", bufs=2))
    f_pool = ctx.enter_context(tc.tile_pool(name="fold", bufs=2))

    # --- constants resident in SBUF ---
    w1t_t = const.tile([ROWS * PACKS, HID * PACKS], BF16)
    nc.sync.dma_start(w1t_t[:], w1t[:])
    b1_t = const.tile([HID * PACKS, 1], F32)
    nc.sync.dma_start(b1_t[:], b1[:])
    w1bt_t = const.tile([HID * PACKS, HID * PACKS], F32)
    nc.sync.dma_start(w1bt_t[:], w1bt[:])
    b1b_t = const.tile([HID * PACKS, 1], F32)
    nc.sync.dma_start(b1b_t[:], b1b[:])
    w2t_t = const.tile([16 * PACKS, HID * PACKS], F32)
    nc.sync.dma_start(w2t_t[:], w2t[:])
    b2_t = const.tile([HID * PACKS, 1], F32)
    nc.sync.dma_start(b2_t[:], b2[:])
    w3t_t = const.tile([HID * PACKS, HID * PACKS], F32)
    nc.sync.dma_start(w3t_t[:], w3t[:])
    b3_t = const.tile([HID * PACKS, 1], F32)
    nc.sync.dma_start(b3_t[:], b3[:])
    invc_t = const.tile([HID * PACKS, NPP], F32)
    nc.sync.dma_start(invc_t[:], invc[:])

    s_full = const.tile([HID * PACKS, NPP], F32)
    mean_full = const.tile([HID * PACKS, NPP], F32, tag="meanf")

    # --- edge phase ---
    rows_per_chunk = (ROWS * PACKS) // dma_chunks
    for blk in range(NBLK):
        rhs = rhs_pool.tile([ROWS * PACKS, BSLOTS], BF16)
        for c in range(dma_chunks):
            r0 = rows_per_chunk * c
            r1 = rows_per_chunk * (c + 1)
            nc.sync.dma_start(
                rhs[r0:r1, :],
                grid[r0:r1, blk * BSLOTS:(blk + 1) * BSLOTS])

        r = r_pool.tile([HID * PACKS, BSLOTS], BF16)
        for i in range(MM_PER_BLK):
            ps = psum_pool.tile([HID * PACKS, MM_F], F32)
            nc.tensor.matmul(
                ps[:], w1t_t[:], rhs[:, i * MM_F:(i + 1) * MM_F],
                start=True, stop=True)
            rdst = r[:, i * MM_F:(i + 1) * MM_F]
            if dve_relu_frac and i % dve_relu_frac == dve_relu_frac - 1:
                nc.vector.tensor_scalar(
                    rdst, ps[:], b1_t[:], 0.0,
                    op0=mybir.AluOpType.add, op1=mybir.AluOpType.max)
            else:
                nc.scalar.activation(
                    rdst, ps[:], mybir.ActivationFunctionType.Relu,
                    bias=b1_t[:], scale=1.0)

        # fold 96 -> 48 -> 24 -> 12 (bf16), then reduce 12 -> 1 (f32)
        r3 = r[:].rearrange("p (n d) -> p n d", d=D)
        f1 = f_pool.tile([HID * PACKS, BS, 48], BF16, tag="f1")
        nc.vector.tensor_tensor(f1[:], r3[:, :, 0:48], r3[:, :, 48:96],
                                op=mybir.AluOpType.add)
        f2 = f_pool.tile([HID * PACKS, BS, 24], BF16, tag="f2")
        nc.vector.tensor_tensor(f2[:], f1[:, :, 0:24], f1[:, :, 24:48],
                                op=mybir.AluOpType.add)
        f3 = f_pool.tile([HID * PACKS, BS, 12], BF16, tag="f3")
        nc.vector.tensor_tensor(f3[:], f2[:, :, 0:12], f2[:, :, 12:24],
                                op=mybir.AluOpType.add)
        nc.vector.tensor_reduce(
            s_full[:, blk * BS:(blk + 1) * BS], f3[:],
            axis=mybir.AxisListType.X, op=mybir.AluOpType.add)

    # --- edge-MLP layer 2 (postponed past the sum):
    #     mean = (S @ w1b) * invc + b1b ---
    segs = [(0, 512), (512, 512), (1024, 512), (1536, 32)]
    for off, sz in segs:
        ps0 = psum_pool.tile([HID * PACKS, MM_F], F32, tag="ps0", bufs=2)
        nc.tensor.matmul(ps0[:, :sz], w1bt_t[:], s_full[:, off:off + sz],
                         start=True, stop=True)
        nc.vector.tensor_tensor(
            mean_full[:, off:off + sz], ps0[:, :sz],
            invc_t[:, off:off + sz], op=mybir.AluOpType.mult)
        nc.scalar.activation(
            mean_full[:, off:off + sz], mean_full[:, off:off + sz],
            mybir.ActivationFunctionType.Identity,
            bias=b1b_t[:], scale=1.0)

    # --- node MLP ---
    rhs2 = const.tile([16 * PACKS, NPP], F32)
    nc.vector.memset(rhs2[15:16 * PACKS:16, :], 1.0)
    nc.sync.dma_start(rhs2[0:16 * PACKS:16, :], qn[:])
    for k in range(PACKS):
        nc.sync.dma_start(
            rhs2[16 * k + 1:16 * k + 15, :],
            mean_full[HID * k:HID * (k + 1), :])

    h = const.tile([HID * PACKS, NPP], F32, tag="h")
    for off, sz in segs:
        ps2 = psum_pool.tile([HID * PACKS, MM_F], F32, tag="ps2", bufs=1)
        nc.tensor.matmul(ps2[:, :sz], w2t_t[:], rhs2[:, off:off + sz],
                         start=True, stop=True)
        nc.scalar.activation(
            h[:, off:off + sz], ps2[:, :sz],
            mybir.ActivationFunctionType.Relu, bias=b2_t[:], scale=1.0)

    osb = const.tile([HID * PACKS, NPP], F32, tag="osb")
    for off, sz in segs:
        ps3 = psum_pool.tile([HID * PACKS, MM_F], F32, tag="ps3", bufs=1)
        nc.tensor.matmul(ps3[:, :sz], w3t_t[:], h[:, off:off + sz],
                         start=True, stop=True)
        nc.vector.tensor_scalar(
            osb[:, off:off + sz], ps3[:, :sz], b3_t[:], None,
            op0=mybir.AluOpType.add)
    nc.sync.dma_start(out[:], osb[:])


def _get_nc():
    if "nc" not in _STATE:
        _STATE["nc"] = _build_nc()
    return _STATE["nc"]


# ============================ host side ============================

def _pad_sentinel(w1a, b1a):
    """Feature vector for empty edge slots: relu(a_pad @ w1a[1:] + b1a
    + 0*w1a[0]) == 0 exactly, robust to bf16 rounding of both a_pad and
    the weights."""
    A = w1a[1:]  # [14, 14]
    t = 3.0e4
    for _ in range(6):
        a_pad = np.linalg.solve(A.T.astype(np.float64),
                                (-t - b1a).astype(np.float64))
        a_bf = _bf16_val(a_pad.astype(np.float32)).astype(np.float32)
        w_bf = _bf16_val(w1a).astype(np.float32)
        u = a_bf @ w_bf[1:] + b1a
        if u.max() < -1.0:
            return a_pad.astype(np.float32)
        t *= 4.0
    raise RuntimeError("could not build relu-zero pad sentinel")


def _edge_mlp_host(q_r, ea, w1a, b1a, w1b, b1b):
    feat = np.concatenate([q_r[:, None], ea], axis=1)
    return np.maximum(feat @ w1a + b1a, 0.0) @ w1b + b1b


def _node_mlp_host(qn, mean, w2a, b2a, w2b, b2b):
    feat = np.concatenate([qn[:, None], mean], axis=1)
    return np.maximum(feat @ w2a + b2a, 0.0) @ w2b + b2b


def _host_reference_rows(nodes, order, starts, counts, q, row, ea,
                         w1a, b1a, w1b, b1b, w2a, b2a, w2b, b2b):
    """Exact rows for a small set of nodes (degree 0 / overflow)."""
    outs = np.empty((len(nodes), HID), np.float32)
    for i, n in enumerate(nodes):
        c = int(counts[n])
        if c == 0:
            mean = np.zeros(HID, np.float32)
        else:
            eidx = order[starts[n]:starts[n] + c]
            h = _edge_mlp_host(q[row[eidx]], ea[eidx],
                               w1a, b1a, w1b, b1b)
            mean = h.mean(axis=0)
        outs[i] = _node_mlp_host(q[n:n + 1], mean[None, :],
                                 w2a, b2a, w2b, b2b)[0]
    return outs


def _numpy_fallback(q, row, col, ea, order, starts, counts,
                    w1a, b1a, w1b, b1b, w2a, b2a, w2b, b2b):
    h = _edge_mlp_host(q[row[order]], ea[order], w1a, b1a, w1b, b1b)
    seg = np.add.reduceat(h, starts[:-1].clip(max=len(order) - 1), axis=0)
    seg[counts == 0] = 0.0
    mean = seg / np.maximum(counts, 1)[:, None]
    return _node_mlp_host(q, mean, w2a, b2a, w2b, b2b)


def kernel(x, edge_index, edge_attr, u, batch,
           w1a, b1a, w1b, b1b, w2a, b2a, w2b, b2b):
    x = np.asarray(x, np.float32)
    ei = np.asarray(edge_index)
    ea = np.asarray(edge_attr, np.float32)
    w1a = np.asarray(w1a, np.float32)
    b1a = np.asarray(b1a, np.float32)
    w1b = np.asarray(w1b, np.float32)
    b1b = np.asarray(b1b, np.float32)
    w2a = np.asarray(w2a, np.float32)
    b2a = np.asarray(b2a, np.float32)
    w2b = np.asarray(w2b, np.float32)
    b2b = np.asarray(b2b, np.float32)
    row = np.ascontiguousarray(ei[0]).astype(np.int64, copy=False)
    col = np.ascontiguousarray(ei[1]).astype(np.int64, copy=False)
    E = row.shape[0]

    # --- index prep: per-edge rank within destination node ---
    q = ((x * _MINK) * x).sum(axis=1)                      # [N] f32
    counts = np.bincount(col, minlength=N)
    starts = np.zeros(N + 1, np.int64)
    np.cumsum(counts, out=starts[1:])
    order = np.argsort(col.astype(np.uint32), kind="stable")
    col_s = col[order]
    rank = np.arange(E, dtype=np.int64) - starts[col_s]

    if x.shape[0] != N:
        return _numpy_fallback(q, row, col, ea, order, starts, counts,
                               w1a, b1a, w1b, b1b, w2a, b2a, w2b, b2b
                               ).astype(np.float32)

    keep = rank < D
    eidx = order[keep]
    dest = col_s[keep]
    rank_k = rank[keep]

    # --- build per-core bf16 grids (uint16 bit views) ---
    a_pad = _pad_sentinel(w1a, b1a)
    G = np.empty((NCORES, PACKS, ROWS, GRID_W), dtype=np.uint16)
    pat = np.zeros(ROWS, np.uint16)
    pat[1:] = _bf16_bits(a_pad)
    G[:] = pat[None, None, :, None]

    core = dest // NPN
    loc = dest % NPN
    base = ((core * PACKS + loc // NPP) * ROWS) * GRID_W \
        + (loc % NPP) * D + rank_k
    GF = G.reshape(-1)
    GF[base] = _bf16_bits(q[row[eidx]])
    ea_k16 = _bf16_bits(ea[eidx])            # [Ek, 14] uint16
    gw = np.int64(GRID_W)
    for f in range(HID):
        GF[base + np.int64(1 + f) * gw] = ea_k16[:, f]
    grid_bf = G.reshape(NCORES, PACKS * ROWS, GRID_W).view(ml_dtypes.bfloat16)

    # --- small per-core tensors ---
    icnt = (1.0 / np.maximum(counts, 1)).astype(np.float32)
    icnt_p = np.ones((NCORES, NLOC), np.float32)
    icnt_p[:, :NPN] = icnt.reshape(NCORES, NPN)
    invc_in = np.ascontiguousarray(
        np.broadcast_to(
            icnt_p.reshape(NCORES, PACKS, 1, NPP),
            (NCORES, PACKS, HID, NPP),
        )
    ).reshape(NCORES, HID * PACKS, NPP)
    qn_p = np.zeros((NCORES, NLOC), np.float32)
    qn_p[:, :NPN] = q.reshape(NCORES, NPN)
    qn_in = qn_p.reshape(NCORES, PACKS, NPP)

    # --- block-diagonal weights ---
    def blockdiag(blk, reps, dtype):
        r, c = blk.shape
        w = np.zeros((r * reps, c * reps), dtype)
        for k in range(reps):
            w[r * k:r * (k + 1), c * k:c * (k + 1)] = blk
        return w

    w1t_in = blockdiag(w1a, PACKS, np.float32)
    w1t_in = _bf16_bits(w1t_in).view(ml_dtypes.bfloat16)
    b1_in = np.tile(b1a, PACKS)[:, None].astype(np.float32)
    w1bt_in = blockdiag(w1b, PACKS, np.float32)
    b1b_in = np.tile(b1b, PACKS)[:, None].astype(np.float32)
    w2t_in = blockdiag(np.vstack([w2a, b2a[None, :]]), PACKS, np.float32)
    b2_in = np.tile(b2a, PACKS)[:, None].astype(np.float32)
    w3t_in = blockdiag(w2b, PACKS, np.float32)
    b3_in = np.tile(b2b, PACKS)[:, None].astype(np.float32)

    in_maps = [
        dict(grid=np.ascontiguousarray(grid_bf[c]), invc=invc_in[c],
             qn=np.ascontiguousarray(qn_in[c]), w1t=w1t_in, b1=b1_in,
             w1bt=w1bt_in, b1b=b1b_in, w2t=w2t_in, b2=b2_in,
             w3t=w3t_in, b3=b3_in)
        for c in range(NCORES)
    ]

    # --- run on the 8 NeuronCores ---
    try:
        from concourse.bass_utils import run_bass_kernel_spmd
        res = run_bass_kernel_spmd(_get_nc(), in_maps,
                                   core_ids=list(range(NCORES)))
        _STATE["last_result"] = res
        parts = []
        for c in range(NCORES):
            o = np.asarray(res.results[c]["out"], np.float32)
            parts.append(
                o.reshape(PACKS, HID, NPP).transpose(0, 2, 1)
                .reshape(NLOC, HID)[:NPN])
        full = np.concatenate(parts, axis=0)
    except Exception:
        import traceback
        traceback.print_exc()
        return _numpy_fallback(q, row, col, ea, order, starts, counts,
                               w1a, b1a, w1b, b1b, w2a, b2a, w2b, b2b
                               ).astype(np.float32)

    # --- exact host rows for degree-0 / overflow nodes ---
    bad = np.flatnonzero((counts > D) | (counts == 0))
    if len(bad):
        full[bad] = _host_reference_rows(
            bad, order, starts, counts, q, row, ea,
            w1a, b1a, w1b, b1b, w2a, b2a, w2b, b2b)

    return full.astype(np.float32)
